# revision 16
# baseline (speedup 1.0000x reference)
"""DARTS mixed-op layer forward on 8 Trainium2 cores — cubic-fold fp16 matmuls.

Math: out[b,j] = sum_{i,k} softmax(alphas,axis=-1)[i,j,k] * coeffs[i,j,k] * prim_k(x[b,i])
with prims = [0, x, x^2, x^3, exp(x), ln(x), 1/x, sin(x)] and x in (0.5, 1.5).

Key restructure: on (0.5, 1.5) each transcendental primitive is replaced by a
least-squares cubic, so the whole per-(i,j) mixture collapses to a single cubic
    f_ij(x) = C0[i,j] + C1[i,j] x + C2[i,j] x^2 + C3[i,j] x^3
folded ON THE HOST from gates*coeffs and the fixed poly coefficients (end-to-end
max-rel error ~3.5e-3 incl. fp16, vs the 2e-2 gate).  The device then only does
    out[b,j] = bias[j] + sum_i sum_{p=1..3} Cp[i,j] * x_i^p
i.e. THREE fp16 matmul channels (x, x^2, x^3): 12288 PE rows/core at 1 cyc/row.

Sharding: batch split across 8 cores (8192 rows each).  Paired layout packs two
128-row batch chunks into the 128 SBUF partitions (p = c*64 + i) and the weights
are block-diagonal diag(Cp, Cp), so one K=128 matmul covers both chunks (1.5 PE
cycles per batch row — the K<=128 floor for a 192-wide contraction).

Schedule notes (from TimelineSim traces): HWDGE descriptor-gen is a shared
serial resource (~630ns per dma_start), so the weights ride in front of the x
tensor in ONE packed dram tensor and x moves in 5 chunked DMAs; x^2/x^3 are
fp16 DVE muls (2x mode) with two late x^3 groups offloaded to gpsimd; PSUM is
evicted by ACT (Identity + per-partition bias, fp16 out) into per-group tiles
(a shared tile would serialize evictions against out-DMA reads); out-DMAs
alternate SP/HWDGE and gpsimd/SWDGE.
"""

import numpy as np

import concourse.bass as bass
import concourse.mybir as mybir
import concourse.tile as tile
from concourse import bacc
from concourse.bass_utils import run_bass_kernel_spmd

F32 = mybir.dt.float32
F16 = mybir.dt.float16
AFT = mybir.ActivationFunctionType

N_CORES = 8
BATCH = 65536
BC = BATCH // N_CORES          # 8192 rows per core
DEG = 3                        # cubic fold
WCOLS = DEG * 128              # weight columns packed ahead of x

# least-squares cubic fits of the transcendental prims on (0.5, 1.5),
# computed once at import time (input-independent).
def _poly_fits(deg: int = DEG) -> dict[int, np.ndarray]:
    grid = np.linspace(0.5, 1.5, 20001)
    V = np.vander(grid, deg + 1, increasing=True)
    fits = {}
    for k, f in ((4, np.exp), (5, np.log), (6, lambda t: 1.0 / t), (7, np.sin)):
        fits[k] = np.linalg.lstsq(V, f(grid), rcond=None)[0]
    return fits

_FITS = _poly_fits()


def build_kernel(bc: int = BC, repeat: int = 1, bufs: int = 2,
                 warm: bool = True, order: str = "blocks") -> bass.Bass:
    fcols = bc // 2            # paired-layout columns (2 batch rows per col)
    ng = fcols // 512          # PSUM banks / matmul col-groups

    nc = bacc.Bacc(None, target_bir_lowering=False, debug=False)
    # xw packs [block-diag weights (384) | bias col (1) | paired x (fcols)]
    xw_d = nc.dram_tensor("xw", [128, WCOLS + 1 + fcols], F16, kind="ExternalInput")
    ot_d = nc.dram_tensor("ot", [128, fcols], F16, kind="ExternalOutput")

    with tile.TileContext(nc) as tc:
        import contextlib

        # scratch pool stays open across the repeat loop; memsets run once
        scr_ctx = tc.tile_pool(name="scr", bufs=1)
        scr = scr_ctx.__enter__()
        wsc = scr.tile([128, 128], F16, name="wsc")
        nc.gpsimd.memset(wsc[:, :], 0.0)
        xsc = scr.tile([128, 512], F16, name="xsc")
        nc.gpsimd.memset(xsc[:, :], 0.0)

        loop_ctx = tc.For_i(0, repeat, 1) if repeat > 1 else contextlib.nullcontext()
        with (
            loop_ctx,
            tc.tile_pool(name="big", bufs=bufs) as big,
            tc.tile_pool(name="small", bufs=bufs) as small,
            tc.tile_pool(name="outp", bufs=bufs) as outp,
            tc.tile_pool(name="psum", bufs=1, space="PSUM") as psum,
        ):
            # ---- packed [weights | bias | x]: 6 chunked DMAs alternating the
            # two HWDGE queues, sized so groups land strictly in processing
            # order on the bandwidth-serial DMA track ----
            XC = WCOLS + 1                 # x column origin in xw
            wx = big.tile([128, XC + fcols], F16, name="wx")
            chunks = [
                (nc.sync, 0, XC + 512),            # A: wd + bias + g0
                (nc.scalar, XC + 512, XC + 1024),  # B: g1
                (nc.sync, XC + 1024, XC + 2048),   # C: g2 g3
                (nc.scalar, XC + 2048, XC + 3072), # D: g4 g5
                (nc.sync, XC + 3072, XC + 3584),   # E: g6
                (nc.scalar, XC + 3584, XC + fcols) # F: g7
            ]
            for e, c0, c1 in chunks:
                e.dma_start(out=wx[:, c0:c1], in_=xw_d[:, c0:c1])

            # fp32 bias for the ACT eviction (cast from the packed fp16 col)
            bs = small.tile([128, 1], F32, name="bs")
            nc.vector.tensor_copy(out=bs[:, :], in_=wx[:, WCOLS:WCOLS + 1])

            def wap(p):                       # stationary weight for channel p
                return wx[:, p * 128:(p + 1) * 128]

            def xap(g0, g1):                  # x cols for groups [g0, g1)
                return wx[:, XC + g0 * 512:XC + g1 * 512]

            x2 = big.tile([128, fcols], F16, name="x2")
            x3 = big.tile([128, fcols], F16, name="x3")
            ps = [psum.tile([128, 512], F32, name=f"ps{g}") for g in range(ng)]

            def mul(eng, dst, a, b, g0, g1):
                c0, c1 = g0 * 512, g1 * 512
                ina = xap(g0, g1) if a is None else a[:, c0:c1]
                inb = xap(g0, g1) if b is None else b[:, c0:c1]
                eng.tensor_mul(out=dst[:, c0:c1], in0=ina, in1=inb)

            # DVE mul stream in arrival order (A:g0, B:g1, C:g2g3, D:g4g5,
            # E:g6, F:g7); squares fuse to 1024-col ops where arrivals allow.
            mul(nc.vector, x2, None, None, 0, 1)
            mul(nc.vector, x3, x2, None, 0, 1)
            mul(nc.vector, x2, None, None, 1, 2)
            mul(nc.vector, x3, x2, None, 1, 2)
            mul(nc.vector, x2, None, None, 2, 4)
            mul(nc.vector, x3, x2, None, 2, 3)
            mul(nc.vector, x3, x2, None, 3, 4)
            mul(nc.vector, x2, None, None, 4, 6)
            mul(nc.vector, x3, x2, None, 4, 5)
            mul(nc.vector, x3, x2, None, 5, 6)
            mul(nc.vector, x2, None, None, 6, 7)
            mul(nc.vector, x3, x2, None, 6, 7)
            mul(nc.vector, x2, None, None, 7, 8)
            mul(nc.vector, x3, x2, None, 7, 8)

            def evict(g, out_eng):
                ob = outp.tile([128, 512], F16, name=f"ob{g}")
                nc.scalar.activation(out=ob[:, :], in_=ps[g][:, :],
                                     func=AFT.Identity, bias=bs[:, 0:1])
                out_eng.dma_start(out=ot_d[:, g * 512:(g + 1) * 512], in_=ob[:, :])

            # PE: warmup matmuls on zero scratch hold the clock at full speed
            # until real data lands (~3.5us); the ladder shrinks near the
            # handoff so at most ~100ns is wasted when real data arrives.
            if warm:
                for cols in (512, 512, 512, 512, 512, 128, 128, 128, 128):
                    nc.tensor.matmul(ps[ng - 1][:, 0:cols], wsc[:, :],
                                     xsc[:, 0:cols], start=True, stop=False)
            # blocks in arrival order; a finished block's banks are evicted
            # while the next block computes.
            out_engs = [nc.sync, nc.scalar, nc.sync, nc.scalar,
                        nc.sync, nc.scalar, nc.sync, nc.scalar]
            if order == "blocks":
                blocks = ((0,), (1,), (2, 3), (4, 5), (6,), (7,))
            else:  # channel-major over all groups: 3 weight loads per pass
                blocks = (tuple(range(ng)),)
            for gs in blocks:
                for p, data in ((0, None), (1, x2), (2, x3)):
                    for g in gs:
                        d = xap(g, g + 1) if data is None else data[:, g * 512:(g + 1) * 512]
                        nc.tensor.matmul(ps[g][:, :], wap(p), d,
                                         start=(p == 0), stop=(p == DEG - 1))
                for g in gs:
                    evict(g, out_engs[g])

        scr_ctx.__exit__(None, None, None)

    nc.compile()
    return nc


_NC_CACHE: dict[int, bass.Bass] = {}


def _get_nc(bc: int = BC) -> bass.Bass:
    if bc not in _NC_CACHE:
        _NC_CACHE[bc] = build_kernel(bc)
    return _NC_CACHE[bc]


def _pair_layout(t: np.ndarray) -> np.ndarray:
    """[bc, 64] fp16 -> [128, bc/2]: out[c*64+i, s*128+b] = t[s*256+c*128+b, i]."""
    nsup = t.shape[0] // 256
    return np.ascontiguousarray(
        t.reshape(nsup, 2, 128, 64).transpose(1, 3, 0, 2).reshape(128, nsup * 128)
    )


def _fold_weights(alphas: np.ndarray, coeffs: np.ndarray):
    """Fold gates*coeffs and the cubic fits into C[p][i,j] (p=0..3)."""
    a = alphas.astype(np.float64)
    e = np.exp(a - a.max(-1, keepdims=True))
    gates = e / e.sum(-1, keepdims=True)
    w = gates * coeffs.astype(np.float64)              # [I, J, K]
    C = np.zeros((DEG + 1, 64, 64))
    for p in (1, 2, 3):                                # exact power channels
        C[p] += w[:, :, p]
    for k, fit in _FITS.items():                       # folded transcendentals
        for p in range(DEG + 1):
            C[p] += w[:, :, k] * fit[p]
    return C


def _prep_inputs(x: np.ndarray, alphas: np.ndarray, coeffs: np.ndarray):
    C = _fold_weights(alphas, coeffs)
    # block-diagonal duplicated weights diag(Cp, Cp), fp16, packed ahead of x
    wd = np.zeros((128, DEG, 128), np.float16)
    for p in (1, 2, 3):
        wd[0:64, p - 1, 0:64] = C[p].astype(np.float16)
        wd[64:128, p - 1, 64:128] = C[p].astype(np.float16)
    wd = wd.reshape(128, WCOLS)
    bias = np.tile(C[0].sum(0), 2).reshape(128, 1).astype(np.float16)

    bc = x.shape[0] // N_CORES
    in_maps = []
    for c in range(N_CORES):
        xs = x[c * bc:(c + 1) * bc].astype(np.float16)
        xw = np.concatenate([wd, bias, _pair_layout(xs)], axis=1)
        in_maps.append({"xw": np.ascontiguousarray(xw)})
    return in_maps, bc


def kernel(x: np.ndarray, alphas: np.ndarray, coeffs: np.ndarray) -> np.ndarray:
    x = np.asarray(x, dtype=np.float32)
    in_maps, bc = _prep_inputs(x, np.asarray(alphas), np.asarray(coeffs))

    nc = _get_nc(bc)
    res = run_bass_kernel_spmd(nc, in_maps, core_ids=list(range(N_CORES)))

    outs = []
    for r in res.results:
        ot = r["ot"].astype(np.float32)                # [128, bc/2]
        nsup = bc // 256
        # ot[c*64+j, s*128+b] -> out[s*256+c*128+b, j]
        outs.append(
            ot.reshape(2, 64, nsup, 128).transpose(2, 0, 3, 1).reshape(bc, 64)
        )
    return np.concatenate(outs, axis=0)


# revision 18
# speedup vs baseline: 1.0945x; 1.0945x over previous
"""DARTS mixed-op layer forward on 8 Trainium2 cores — cubic-fold fp16 matmuls.

Math: out[b,j] = sum_{i,k} softmax(alphas,axis=-1)[i,j,k] * coeffs[i,j,k] * prim_k(x[b,i])
with prims = [0, x, x^2, x^3, exp(x), ln(x), 1/x, sin(x)] and x in (0.5, 1.5).

Key restructure: on (0.5, 1.5) each transcendental primitive is replaced by a
least-squares cubic, so the whole per-(i,j) mixture collapses to a single cubic
    f_ij(x) = C0[i,j] + C1[i,j] x + C2[i,j] x^2 + C3[i,j] x^3
folded ON THE HOST from gates*coeffs and the fixed poly coefficients (end-to-end
max-rel error ~3.5e-3 incl. fp16, vs the 2e-2 gate).  The device then only does
    out[b,j] = bias[j] + sum_i sum_{p=1..3} Cp[i,j] * x_i^p
i.e. THREE fp16 matmul channels (x, x^2, x^3): 12288 PE rows/core at 1 cyc/row.

Sharding: batch split across 8 cores (8192 rows each).  Paired layout packs two
128-row batch chunks into the 128 SBUF partitions (p = c*64 + i) and the weights
are block-diagonal diag(Cp, Cp), so one K=128 matmul covers both chunks (1.5 PE
cycles per batch row — the K<=128 floor for a 192-wide contraction).

Schedule notes (from TimelineSim traces): HWDGE descriptor-gen is a shared
serial resource (~630ns per dma_start), so the weights ride in front of the x
tensor in ONE packed dram tensor and x moves in 5 chunked DMAs; x^2/x^3 are
fp16 DVE muls (2x mode) with two late x^3 groups offloaded to gpsimd; PSUM is
evicted by ACT (Identity + per-partition bias, fp16 out) into per-group tiles
(a shared tile would serialize evictions against out-DMA reads); out-DMAs
alternate SP/HWDGE and gpsimd/SWDGE.
"""

import numpy as np

import concourse.bass as bass
import concourse.mybir as mybir
import concourse.tile as tile
from concourse import bacc
from concourse.bass_utils import run_bass_kernel_spmd

F32 = mybir.dt.float32
F16 = mybir.dt.float16
AFT = mybir.ActivationFunctionType

N_CORES = 8
BATCH = 65536
BC = BATCH // N_CORES          # 8192 rows per core
DEG = 3                        # cubic fold
WCOLS = DEG * 128              # weight columns packed ahead of x

# least-squares cubic fits of the transcendental prims on (0.5, 1.5),
# computed once at import time (input-independent).
def _poly_fits(deg: int = DEG) -> dict[int, np.ndarray]:
    grid = np.linspace(0.5, 1.5, 20001)
    V = np.vander(grid, deg + 1, increasing=True)
    fits = {}
    for k, f in ((4, np.exp), (5, np.log), (6, lambda t: 1.0 / t), (7, np.sin)):
        fits[k] = np.linalg.lstsq(V, f(grid), rcond=None)[0]
    return fits

_FITS = _poly_fits()


def build_kernel(bc: int = BC, repeat: int = 1, bufs: int = 2,
                 warm: bool = True, order: str = "blocks") -> bass.Bass:
    fcols = bc // 2            # paired-layout columns (2 batch rows per col)
    ng = fcols // 512          # PSUM banks / matmul col-groups

    nc = bacc.Bacc(None, target_bir_lowering=False, debug=False)
    # xw packs [block-diag weights (384) | bias col (1) | paired x (fcols)]
    xw_d = nc.dram_tensor("xw", [128, WCOLS + 1 + fcols], F16, kind="ExternalInput")
    ot_d = nc.dram_tensor("ot", [128, fcols], F16, kind="ExternalOutput")

    with tile.TileContext(nc) as tc:
        import contextlib

        # scratch pool stays open across the repeat loop; memsets run once
        scr_ctx = tc.tile_pool(name="scr", bufs=1)
        scr = scr_ctx.__enter__()
        wsc = scr.tile([128, 128], F16, name="wsc")
        nc.gpsimd.memset(wsc[:, :], 0.0)
        xsc = scr.tile([128, 512], F16, name="xsc")
        nc.gpsimd.memset(xsc[:, :], 0.0)

        loop_ctx = tc.For_i(0, repeat, 1) if repeat > 1 else contextlib.nullcontext()
        with (
            loop_ctx,
            tc.tile_pool(name="big", bufs=bufs) as big,
            tc.tile_pool(name="small", bufs=bufs) as small,
            tc.tile_pool(name="outp", bufs=bufs) as outp,
            tc.tile_pool(name="psum", bufs=1, space="PSUM") as psum,
        ):
            # ---- packed [weights | bias | x]: chunked DMAs alternating the
            # two HWDGE queues, sized so groups land strictly in processing
            # order on the bandwidth-serial DMA track.  The "wide" order uses
            # 2 fat chunks instead (fewer descriptor-gens per loop pass). ----
            XC = WCOLS + 1                 # x column origin in xw
            wx = big.tile([128, XC + fcols], F16, name="wx")
            if order == "wide":
                chunks = [
                    (nc.sync, 0, XC + 2048),
                    (nc.scalar, XC + 2048, XC + fcols),
                ]
            else:
                chunks = [
                    (nc.sync, 0, XC + 512),            # A: wd + bias + g0
                    (nc.scalar, XC + 512, XC + 1024),  # B: g1
                    (nc.sync, XC + 1024, XC + 2048),   # C: g2 g3
                    (nc.scalar, XC + 2048, XC + 3072), # D: g4 g5
                    (nc.sync, XC + 3072, XC + 3584),   # E: g6
                    (nc.scalar, XC + 3584, XC + fcols) # F: g7
                ]
            for e, c0, c1 in chunks:
                e.dma_start(out=wx[:, c0:c1], in_=xw_d[:, c0:c1])

            # fp32 bias for the ACT eviction (cast from the packed fp16 col)
            bs = small.tile([128, 1], F32, name="bs")
            nc.vector.tensor_copy(out=bs[:, :], in_=wx[:, WCOLS:WCOLS + 1])

            def wap(p):                       # stationary weight for channel p
                return wx[:, p * 128:(p + 1) * 128]

            def xap(g0, g1):                  # x cols for groups [g0, g1)
                return wx[:, XC + g0 * 512:XC + g1 * 512]

            x2 = big.tile([128, fcols], F16, name="x2")
            x3 = big.tile([128, fcols], F16, name="x3")
            ps = [psum.tile([128, 512], F32, name=f"ps{g}") for g in range(ng)]

            def mul(eng, dst, a, b, g0, g1):
                c0, c1 = g0 * 512, g1 * 512
                ina = xap(g0, g1) if a is None else a[:, c0:c1]
                inb = xap(g0, g1) if b is None else b[:, c0:c1]
                eng.tensor_mul(out=dst[:, c0:c1], in0=ina, in1=inb)

            # DVE mul stream in arrival order; squares fuse to 1024-col ops
            # where arrivals allow.
            if order == "wide":
                for h in range(ng // 2):
                    mul(nc.vector, x2, None, None, 2 * h, 2 * h + 2)
                    mul(nc.vector, x3, x2, None, 2 * h, 2 * h + 2)
            else:
                mul(nc.vector, x2, None, None, 0, 1)
                mul(nc.vector, x3, x2, None, 0, 1)
                mul(nc.vector, x2, None, None, 1, 2)
                mul(nc.vector, x3, x2, None, 1, 2)
                mul(nc.vector, x2, None, None, 2, 4)
                mul(nc.vector, x3, x2, None, 2, 3)
                mul(nc.vector, x3, x2, None, 3, 4)
                mul(nc.vector, x2, None, None, 4, 6)
                mul(nc.vector, x3, x2, None, 4, 5)
                mul(nc.vector, x3, x2, None, 5, 6)
                mul(nc.vector, x2, None, None, 6, 7)
                mul(nc.vector, x3, x2, None, 6, 7)
                mul(nc.vector, x2, None, None, 7, 8)
                mul(nc.vector, x3, x2, None, 7, 8)

            def evict(g, ob, col):
                nc.scalar.activation(out=ob[:, col:col + 512], in_=ps[g][:, :],
                                     func=AFT.Identity, bias=bs[:, 0:1])

            # PE: warmup matmuls on zero scratch hold the clock at full speed
            # until real data lands (~3.5us); the ladder shrinks near the
            # handoff so at most ~100ns is wasted when real data arrives.
            if warm:
                for cols in (512, 512, 512, 512, 512, 128, 128, 128, 128):
                    nc.tensor.matmul(ps[ng - 1][:, 0:cols], wsc[:, :],
                                     xsc[:, 0:cols], start=True, stop=False)
            # blocks in arrival order; a finished block's banks are evicted
            # while the next block computes.
            if order == "wide":
                blocks = (tuple(range(0, ng // 2)), tuple(range(ng // 2, ng)))
            elif order == "blocks":
                blocks = ((0,), (1,), (2, 3), (4, 5), (6,), (7,))
            else:  # channel-major over all groups: 3 weight loads per pass
                blocks = (tuple(range(ng)),)
            out_engs = [nc.sync, nc.scalar, nc.sync, nc.scalar,
                        nc.sync, nc.scalar, nc.sync, nc.scalar]
            for bi, gs in enumerate(blocks):
                for p, data in ((0, None), (1, x2), (2, x3)):
                    for g in gs:
                        d = xap(g, g + 1) if data is None else data[:, g * 512:(g + 1) * 512]
                        nc.tensor.matmul(ps[g][:, :], wap(p), d,
                                         start=(p == 0), stop=(p == DEG - 1))
                ob = outp.tile([128, 512 * len(gs)], F16, name=f"ob{bi}")
                for ci, g in enumerate(gs):
                    evict(g, ob, ci * 512)
                eng = out_engs[gs[0]]
                eng.dma_start(out=ot_d[:, gs[0] * 512:(gs[-1] + 1) * 512],
                              in_=ob[:, :])

        scr_ctx.__exit__(None, None, None)

    nc.compile()
    return nc


_NC_CACHE: dict[int, bass.Bass] = {}


def _get_nc(bc: int = BC) -> bass.Bass:
    if bc not in _NC_CACHE:
        _NC_CACHE[bc] = build_kernel(bc)
    return _NC_CACHE[bc]


def _pair_layout(t: np.ndarray) -> np.ndarray:
    """[bc, 64] fp16 -> [128, bc/2]: out[c*64+i, s*128+b] = t[s*256+c*128+b, i]."""
    nsup = t.shape[0] // 256
    return np.ascontiguousarray(
        t.reshape(nsup, 2, 128, 64).transpose(1, 3, 0, 2).reshape(128, nsup * 128)
    )


def _fold_weights(alphas: np.ndarray, coeffs: np.ndarray):
    """Fold gates*coeffs and the cubic fits into C[p][i,j] (p=0..3)."""
    a = alphas.astype(np.float64)
    e = np.exp(a - a.max(-1, keepdims=True))
    gates = e / e.sum(-1, keepdims=True)
    w = gates * coeffs.astype(np.float64)              # [I, J, K]
    C = np.zeros((DEG + 1, 64, 64))
    for p in (1, 2, 3):                                # exact power channels
        C[p] += w[:, :, p]
    for k, fit in _FITS.items():                       # folded transcendentals
        for p in range(DEG + 1):
            C[p] += w[:, :, k] * fit[p]
    return C


def _prep_inputs(x: np.ndarray, alphas: np.ndarray, coeffs: np.ndarray):
    C = _fold_weights(alphas, coeffs)
    # block-diagonal duplicated weights diag(Cp, Cp), fp16, packed ahead of x
    wd = np.zeros((128, DEG, 128), np.float16)
    for p in (1, 2, 3):
        wd[0:64, p - 1, 0:64] = C[p].astype(np.float16)
        wd[64:128, p - 1, 64:128] = C[p].astype(np.float16)
    wd = wd.reshape(128, WCOLS)
    bias = np.tile(C[0].sum(0), 2).reshape(128, 1).astype(np.float16)

    bc = x.shape[0] // N_CORES
    in_maps = []
    for c in range(N_CORES):
        xs = x[c * bc:(c + 1) * bc].astype(np.float16)
        xw = np.concatenate([wd, bias, _pair_layout(xs)], axis=1)
        in_maps.append({"xw": np.ascontiguousarray(xw)})
    return in_maps, bc


def kernel(x: np.ndarray, alphas: np.ndarray, coeffs: np.ndarray) -> np.ndarray:
    x = np.asarray(x, dtype=np.float32)
    in_maps, bc = _prep_inputs(x, np.asarray(alphas), np.asarray(coeffs))

    nc = _get_nc(bc)
    res = run_bass_kernel_spmd(nc, in_maps, core_ids=list(range(N_CORES)))

    outs = []
    for r in res.results:
        ot = r["ot"].astype(np.float32)                # [128, bc/2]
        nsup = bc // 256
        # ot[c*64+j, s*128+b] -> out[s*256+c*128+b, j]
        outs.append(
            ot.reshape(2, 64, nsup, 128).transpose(2, 0, 3, 1).reshape(bc, 64)
        )
    return np.concatenate(outs, axis=0)


# revision 20
# speedup vs baseline: 1.2218x; 1.1163x over previous
"""DARTS mixed-op layer forward on 8 Trainium2 cores — cubic-fold fp16 matmuls.

Math: out[b,j] = sum_{i,k} softmax(alphas,axis=-1)[i,j,k] * coeffs[i,j,k] * prim_k(x[b,i])
with prims = [0, x, x^2, x^3, exp(x), ln(x), 1/x, sin(x)] and x in (0.5, 1.5).

Key restructure: on (0.5, 1.5) each transcendental primitive is replaced by a
least-squares cubic, so the whole per-(i,j) mixture collapses to a single cubic
    f_ij(x) = C0[i,j] + C1[i,j] x + C2[i,j] x^2 + C3[i,j] x^3
folded ON THE HOST from gates*coeffs and the fixed poly coefficients (end-to-end
max-rel error ~3.5e-3 incl. fp16, vs the 2e-2 gate).  The device then only does
    out[b,j] = bias[j] + sum_i sum_{p=1..3} Cp[i,j] * x_i^p
i.e. THREE fp16 matmul channels (x, x^2, x^3): 12288 PE rows/core at 1 cyc/row.

Sharding: batch split across 8 cores (8192 rows each).  Paired layout packs two
128-row batch chunks into the 128 SBUF partitions (p = c*64 + i) and the weights
are block-diagonal diag(Cp, Cp), so one K=128 matmul covers both chunks (1.5 PE
cycles per batch row — the K<=128 floor for a 192-wide contraction).

Schedule notes (from TimelineSim traces): HWDGE descriptor-gen is a shared
serial resource (~630ns per dma_start), so the weights ride in front of the x
tensor in ONE packed dram tensor and x moves in 5 chunked DMAs; x^2/x^3 are
fp16 DVE muls (2x mode) with two late x^3 groups offloaded to gpsimd; PSUM is
evicted by ACT (Identity + per-partition bias, fp16 out) into per-group tiles
(a shared tile would serialize evictions against out-DMA reads); out-DMAs
alternate SP/HWDGE and gpsimd/SWDGE.
"""

import numpy as np

import concourse.bass as bass
import concourse.mybir as mybir
import concourse.tile as tile
from concourse import bacc
from concourse.bass_utils import run_bass_kernel_spmd

F32 = mybir.dt.float32
F16 = mybir.dt.float16
AFT = mybir.ActivationFunctionType

N_CORES = 8
BATCH = 65536
BC = BATCH // N_CORES          # 8192 rows per core
DEG = 3                        # cubic fold
WCOLS = DEG * 128              # weight columns packed ahead of x

# least-squares cubic fits of the transcendental prims on (0.5, 1.5),
# computed once at import time (input-independent).
def _poly_fits(deg: int = DEG) -> dict[int, np.ndarray]:
    grid = np.linspace(0.5, 1.5, 20001)
    V = np.vander(grid, deg + 1, increasing=True)
    fits = {}
    for k, f in ((4, np.exp), (5, np.log), (6, lambda t: 1.0 / t), (7, np.sin)):
        fits[k] = np.linalg.lstsq(V, f(grid), rcond=None)[0]
    return fits

_FITS = _poly_fits()


WARM_LADDER = (512, 512, 512, 512, 512, 128, 128, 128, 128)


def build_kernel(bc: int = BC, repeat: int = 1, bufs: int = 2,
                 warm: tuple = WARM_LADDER, order: str = "blocks") -> bass.Bass:
    fcols = bc // 2            # paired-layout columns (2 batch rows per col)
    ng = fcols // 512          # PSUM banks / matmul col-groups

    nc = bacc.Bacc(None, target_bir_lowering=False, debug=False)
    # xw packs [block-diag weights (384) | bias col (1) | paired x (fcols)]
    xw_d = nc.dram_tensor("xw", [128, WCOLS + 1 + fcols], F16, kind="ExternalInput")
    ot_d = nc.dram_tensor("ot", [128, fcols], F16, kind="ExternalOutput")

    with tile.TileContext(nc) as tc:
        import contextlib

        # scratch pool stays open across the repeat loop; memsets run once
        scr_ctx = tc.tile_pool(name="scr", bufs=1)
        scr = scr_ctx.__enter__()
        wsc = scr.tile([128, 128], F16, name="wsc")
        nc.gpsimd.memset(wsc[:, :], 0.0)
        xsc = scr.tile([128, 512], F16, name="xsc")
        nc.gpsimd.memset(xsc[:, :], 0.0)

        loop_ctx = tc.For_i(0, repeat, 1) if repeat > 1 else contextlib.nullcontext()
        with (
            loop_ctx,
            tc.tile_pool(name="big", bufs=bufs) as big,
            tc.tile_pool(name="small", bufs=bufs) as small,
            tc.tile_pool(name="outp", bufs=bufs) as outp,
            tc.tile_pool(name="psum", bufs=1, space="PSUM") as psum,
        ):
            # ---- packed [weights | bias | x]: chunked DMAs alternating the
            # two HWDGE queues, sized so groups land strictly in processing
            # order on the bandwidth-serial DMA track.  The "wide" order uses
            # 2 fat chunks instead (fewer descriptor-gens per loop pass). ----
            XC = WCOLS + 1                 # x column origin in xw
            wx = big.tile([128, XC + fcols], F16, name="wx")
            if order == "wide":
                chunks = [
                    (nc.sync, 0, XC + 2048),
                    (nc.scalar, XC + 2048, XC + fcols),
                ]
            else:
                chunks = [
                    (nc.sync, 0, XC + 512),            # A: wd + bias + g0
                    (nc.scalar, XC + 512, XC + 1024),  # B: g1
                    (nc.sync, XC + 1024, XC + 2048),   # C: g2 g3
                    (nc.scalar, XC + 2048, XC + 3072), # D: g4 g5
                    (nc.sync, XC + 3072, XC + 3584),   # E: g6
                    (nc.scalar, XC + 3584, XC + fcols) # F: g7
                ]
            for e, c0, c1 in chunks:
                e.dma_start(out=wx[:, c0:c1], in_=xw_d[:, c0:c1])

            # fp32 bias for the ACT eviction (cast from the packed fp16 col)
            bs = small.tile([128, 1], F32, name="bs")
            nc.vector.tensor_copy(out=bs[:, :], in_=wx[:, WCOLS:WCOLS + 1])

            def wap(p):                       # stationary weight for channel p
                return wx[:, p * 128:(p + 1) * 128]

            def xap(g0, g1):                  # x cols for groups [g0, g1)
                return wx[:, XC + g0 * 512:XC + g1 * 512]

            x2 = big.tile([128, fcols], F16, name="x2")
            x3 = big.tile([128, fcols], F16, name="x3")
            ps = [psum.tile([128, 512], F32, name=f"ps{g}") for g in range(ng)]

            def mul(eng, dst, a, b, g0, g1):
                c0, c1 = g0 * 512, g1 * 512
                ina = xap(g0, g1) if a is None else a[:, c0:c1]
                inb = xap(g0, g1) if b is None else b[:, c0:c1]
                eng.tensor_mul(out=dst[:, c0:c1], in0=ina, in1=inb)

            # DVE mul stream in arrival order; squares fuse to 1024-col ops
            # where arrivals allow.
            if order == "wide":
                for h in range(ng // 2):
                    mul(nc.vector, x2, None, None, 2 * h, 2 * h + 2)
                    mul(nc.vector, x3, x2, None, 2 * h, 2 * h + 2)
            else:
                mul(nc.vector, x2, None, None, 0, 1)
                mul(nc.vector, x3, x2, None, 0, 1)
                mul(nc.vector, x2, None, None, 1, 2)
                mul(nc.vector, x3, x2, None, 1, 2)
                mul(nc.vector, x2, None, None, 2, 4)
                mul(nc.vector, x3, x2, None, 2, 3)
                mul(nc.vector, x3, x2, None, 3, 4)
                mul(nc.vector, x2, None, None, 4, 6)
                mul(nc.vector, x3, x2, None, 4, 5)
                mul(nc.vector, x3, x2, None, 5, 6)
                mul(nc.vector, x2, None, None, 6, 7)
                mul(nc.vector, x3, x2, None, 6, 7)
                mul(nc.vector, x2, None, None, 7, 8)
                mul(nc.vector, x3, x2, None, 7, 8)

            def evict(g, ob, col):
                nc.scalar.activation(out=ob[:, col:col + 512], in_=ps[g][:, :],
                                     func=AFT.Identity, bias=bs[:, 0:1])

            # PE: warmup matmuls on zero scratch hold the clock at full speed
            # until real data lands (~3.5us); the ladder shrinks near the
            # handoff so at most ~100ns is wasted when real data arrives.
            for cols in (warm or ()):
                nc.tensor.matmul(ps[ng - 1][:, 0:cols], wsc[:, :],
                                 xsc[:, 0:cols], start=True, stop=False)
            # blocks in arrival order; a finished block's banks are evicted
            # while the next block computes.
            if order == "wide":
                blocks = (tuple(range(0, ng // 2)), tuple(range(ng // 2, ng)))
            elif order == "blocks":
                blocks = ((0,), (1,), (2, 3), (4, 5), (6,), (7,))
            else:  # channel-major over all groups: 3 weight loads per pass
                blocks = (tuple(range(ng)),)
            out_engs = [nc.sync, nc.scalar, nc.sync, nc.scalar,
                        nc.sync, nc.scalar, nc.sync, nc.scalar]
            for bi, gs in enumerate(blocks):
                for p, data in ((0, None), (1, x2), (2, x3)):
                    for g in gs:
                        d = xap(g, g + 1) if data is None else data[:, g * 512:(g + 1) * 512]
                        nc.tensor.matmul(ps[g][:, :], wap(p), d,
                                         start=(p == 0), stop=(p == DEG - 1))
                ob = outp.tile([128, 512 * len(gs)], F16, name=f"ob{bi}")
                for ci, g in enumerate(gs):
                    evict(g, ob, ci * 512)
                eng = out_engs[gs[0]]
                eng.dma_start(out=ot_d[:, gs[0] * 512:(gs[-1] + 1) * 512],
                              in_=ob[:, :])

        scr_ctx.__exit__(None, None, None)

    nc.compile()
    return nc


_NC_CACHE: dict[int, bass.Bass] = {}


def _get_nc(bc: int = BC) -> bass.Bass:
    if bc not in _NC_CACHE:
        _NC_CACHE[bc] = build_kernel(bc)
    return _NC_CACHE[bc]


def _pair_layout(t: np.ndarray) -> np.ndarray:
    """[bc, 64] fp16 -> [128, bc/2]: out[c*64+i, s*128+b] = t[s*256+c*128+b, i]."""
    nsup = t.shape[0] // 256
    return np.ascontiguousarray(
        t.reshape(nsup, 2, 128, 64).transpose(1, 3, 0, 2).reshape(128, nsup * 128)
    )


def _fold_weights(alphas: np.ndarray, coeffs: np.ndarray):
    """Fold gates*coeffs and the cubic fits into C[p][i,j] (p=0..3)."""
    a = alphas.astype(np.float64)
    e = np.exp(a - a.max(-1, keepdims=True))
    gates = e / e.sum(-1, keepdims=True)
    w = gates * coeffs.astype(np.float64)              # [I, J, K]
    C = np.zeros((DEG + 1, 64, 64))
    for p in (1, 2, 3):                                # exact power channels
        C[p] += w[:, :, p]
    for k, fit in _FITS.items():                       # folded transcendentals
        for p in range(DEG + 1):
            C[p] += w[:, :, k] * fit[p]
    return C


def _prep_inputs(x: np.ndarray, alphas: np.ndarray, coeffs: np.ndarray):
    C = _fold_weights(alphas, coeffs)
    # block-diagonal duplicated weights diag(Cp, Cp), fp16, packed ahead of x
    wd = np.zeros((128, DEG, 128), np.float16)
    for p in (1, 2, 3):
        wd[0:64, p - 1, 0:64] = C[p].astype(np.float16)
        wd[64:128, p - 1, 64:128] = C[p].astype(np.float16)
    wd = wd.reshape(128, WCOLS)
    bias = np.tile(C[0].sum(0), 2).reshape(128, 1).astype(np.float16)

    bc = x.shape[0] // N_CORES
    in_maps = []
    for c in range(N_CORES):
        xs = x[c * bc:(c + 1) * bc].astype(np.float16)
        xw = np.concatenate([wd, bias, _pair_layout(xs)], axis=1)
        in_maps.append({"xw": np.ascontiguousarray(xw)})
    return in_maps, bc


def kernel(x: np.ndarray, alphas: np.ndarray, coeffs: np.ndarray) -> np.ndarray:
    x = np.asarray(x, dtype=np.float32)
    in_maps, bc = _prep_inputs(x, np.asarray(alphas), np.asarray(coeffs))

    nc = _get_nc(bc)
    res = run_bass_kernel_spmd(nc, in_maps, core_ids=list(range(N_CORES)))

    outs = []
    for r in res.results:
        ot = r["ot"].astype(np.float32)                # [128, bc/2]
        nsup = bc // 256
        # ot[c*64+j, s*128+b] -> out[s*256+c*128+b, j]
        outs.append(
            ot.reshape(2, 64, nsup, 128).transpose(2, 0, 3, 1).reshape(bc, 64)
        )
    return np.concatenate(outs, axis=0)
